# revision 1
# baseline (speedup 1.0000x reference)
"""Trainium2 Bass kernel for nn_CanadarmJacob (space-arm Jacobian, bm=1 path).

Contract: kernel(**inputs) takes FULL inputs (com_list (512,256,3,7) f32,
link_pose_list (512,256,4,4,9) f32, bm scalar) and returns the FULL output
(512,256,6,7) f32. Internally shards samples across 8 NeuronCores (pure data
parallel), runs a Bass/Tile kernel per core, and gathers.

Math (reformulated from the reference):
  pos   = pose[:3, 3, :7]
  rot   = pose[:3, AXIS[a], a] with AXIS=[2,0,2,2,2,0,2], rot[:,4] *= -1
  del   = com - pos
  jacob = rot x del                          (per-act cross product)
  w_k   = sum_{a>=k} M_a del_a               (suffix mass-weighted cumsum)
  Hphi  = D_suf ⊙ rot + w x jacob            (D_suf = suffix inertia diags)
  S_cc' = sum_a M_a del_c del_c'             (6 unique entries)
  c     = (sum_a M_a com_a)/TM - BASE
  H_s   = TM(c c^T - |c|^2 I) + CD + (Sxx+Syy+Szz) I - S
  jsm   = SM_k jacob_k                       (SM = suffix masses)
  Hth   = Hphi - c x jsm
  A     = -inv(H_s)   (symmetric 3x3, via adjugate and ACT reciprocal)
  bot   = A @ Hth
  top   = -(1/TM) jsm + c x bot
  out   = concat(top, bot) rows
"""
import sys
import functools

if "/opt/trn_rl_repo" not in sys.path:
    sys.path.insert(0, "/opt/trn_rl_repo")

import numpy as np

# ---------------------------------------------------------------- constants
N_CORES = 8
P = 128          # SBUF partitions
J = 128          # samples per partition per core
S_CORE = P * J   # 16384 samples per core
N_ACT = 7

MASS = np.array([105.98, 105.98, 314.98, 279.2, 105.98, 105.98, 243.66], np.float64)
TM = float(MASS.sum() + 100000.0 + 243.66)
DIAGS = np.array([[12.19, 12.19, 3.061], [12.19, 12.19, 3.061], [15.41, 2094.71, 2103.19],
                  [9.522, 1966.28, 1966.28], [8.305, 3.061, 8.0386], [12.13, 12.13, 3.061],
                  [9.336, 44.41, 44.41]], np.float64)
D_SUF = np.cumsum(DIAGS[::-1], axis=0)[::-1]          # (7,3) suffix inertia diag
SM = np.cumsum(MASS[::-1])[::-1]                      # (7,) suffix mass
CD = DIAGS.sum(axis=0)                                # (3,)
_TF0 = np.array([[1, 0, 0, 0], [0, -1, 0, 0], [0, 0, 1.3, 6], [0, 0, 0, 1]], np.float64)
_COM0 = np.array([[1, 0, 0, 0], [0, 1, 0, 0], [0, 0, 1, 0.5], [0, 0, 0, 1]], np.float64)
BASE = (_TF0 @ _COM0)[:3, 3] * 243.66 / (100000.0 + 243.66)   # [0, 0, ~0.0162]

# consts row layout (replicated to all 128 partitions host-side)
#   [0:7] M, [7:14] SM, [14:35] D (c-major: D[c][a]), [35:38] CD
CONSTS = np.concatenate([MASS, SM, D_SUF.T.reshape(-1), CD]).astype(np.float32)
NCONST = CONSTS.shape[0]

# smalls tile row indices (each row is (128, J) f32)
SS_R, CSQ_R = 0, 1
CC = 2            # rows 2..4 diag (xx,yy,zz), 5..7 off (xy,xz,yz)
HS = 8            # rows 8..13: [xx,yy,zz,xy,xz,yz]
ADJ = 14          # rows 14..19: [a11,a22,a33,a12,a13,a23]
M1_R, M2_R = 20, 22
T0_R, T1_R, T2_R = 24, 25, 26
DET_R, RDN_R = 27, 28
NSMALL = 29


def _emit(nc, tc, ctx, dram):
    import concourse.bass as bass
    from concourse import mybir

    f32 = mybir.dt.float32
    OP = mybir.AluOpType
    AX = mybir.AxisListType
    V = nc.vector
    G = nc.gpsimd

    NCH = 4                       # pose DMA / early-stage pipeline chunks
    CJ = J // NCH

    pool = ctx.enter_context(tc.tile_pool(name="main", bufs=1))
    ppool = ctx.enter_context(tc.tile_pool(name="pp", bufs=NCH))

    consts = pool.tile([P, NCONST], f32)
    # pose chunks + outb rotate through the same NCH slots (sized by outb)
    poses = [ppool.tile([P, CJ, 144], f32, tag="px", name=f"pose{h}")
             for h in range(NCH)]
    com = pool.tile([P, J, 21], f32, tag="com")
    delb = pool.tile([P, 3, J, N_ACT], f32, tag="dj")
    mw = pool.tile([P, 3, J, N_ACT], f32)     # mdel, suffix-summed in place -> w
    jac = pool.tile([P, 3, J, N_ACT], f32)
    hphi = pool.tile([P, 3, J, N_ACT], f32)   # Hphi -> Htheta in place
    prod = pool.tile([P, 9, J, N_ACT], f32)   # 6 S-products + 3 mcom; later scratch
    cv = pool.tile([P, 3, J], f32)
    smalls = pool.tile([P, NSMALL, J], f32)
    outb = ppool.tile([P, J, 42], f32, tag="px")    # rotates onto pose slots
    red = pool.tile([P, 9, J], f32, tag="com")      # reuses com slot

    # All input DMAs on the sync HWDGE ring (FIFO). consts+com are small and
    # gate the first del chunk, so they go first; pose chunks follow and the
    # chunk-h compute pipelines behind them.
    nc.sync.dma_start(out=consts[:], in_=dram["consts"][:])
    nc.sync.dma_start(out=com[:], in_=dram["com"][:])
    for h in range(NCH):
        nc.sync.dma_start(out=poses[h][:],
                          in_=dram["pose"][:, h * CJ:(h + 1) * CJ, :])

    # handy views
    comR = com[:].rearrange("p j (c a) -> p c j a", c=3)          # (P,3,J,7)

    def bc(ap, shape):
        return ap.broadcast_to(shape)

    Mb = bc(consts[:, 0:7].unsqueeze(1).unsqueeze(2), (P, 3, CJ, N_ACT))
    SMb = bc(consts[:, 7:14].unsqueeze(1).unsqueeze(2), (P, 3, J, N_ACT))
    Db = bc(consts[:, 14:35].rearrange("p (c a) -> p c a", c=3).unsqueeze(2),
            (P, 3, J, N_ACT))
    CDb = bc(consts[:, 35:38].unsqueeze(2), (P, 3, J))

    # early stages per pose chunk, pipelined behind the chunk DMAs
    rots = []
    for h in range(NCH):
        js = slice(h * CJ, (h + 1) * CJ)
        poseR = poses[h][:].rearrange("p j (r k) -> p r j k", r=4)[:, 0:3]
        # rot = axis-col-2 block patched in place: acts 1,5 from axis col 0,
        # act 4 sign-flipped. No gather copy needed.  poseR: (P,3,CJ,36)
        V.tensor_copy(out=poseR[:, :, :, 19:24:4], in_=poseR[:, :, :, 1:6:4])
        V.tensor_scalar_mul(poseR[:, :, :, 22], poseR[:, :, :, 22], -1.0)
        rot = poseR[:, :, :, 18:25]
        rots.append(rot)
        posV = poseR[:, :, :, 27:34]
        dl = delb[:, :, js]
        V.tensor_tensor(out=dl, in0=comR[:, :, js], in1=posV, op=OP.subtract)
        V.tensor_tensor(out=mw[:, :, js], in0=Mb, in1=dl, op=OP.mult)
        for k, (i, j) in enumerate([(0, 0), (1, 1), (2, 2), (0, 1), (0, 2), (1, 2)]):
            V.tensor_tensor(out=prod[:, k, js], in0=mw[:, i, js], in1=delb[:, j, js],
                            op=OP.mult)
        V.tensor_tensor(out=prod[:, 6:9, js], in0=Mb, in1=comR[:, :, js], op=OP.mult)
        for cx in range(3):
            y, z = (cx + 1) % 3, (cx + 2) % 3
            V.tensor_tensor(out=hphi[:, y, js], in0=rot[:, y], in1=delb[:, z, js],
                            op=OP.mult)
            V.tensor_tensor(out=hphi[:, z, js], in0=rot[:, z], in1=delb[:, y, js],
                            op=OP.mult)
            V.tensor_tensor(out=jac[:, cx, js], in0=hphi[:, y, js],
                            in1=hphi[:, z, js], op=OP.subtract)
        # act-reduction of the 9 product rows for this chunk
        V.tensor_reduce(out=red[:, :, js], in_=prod[:, :, js], axis=AX.X, op=OP.add)

    tu = prod[:, 0:3]
    tv = prod[:, 3:6]

    # c = scom/TM - BASE   (BASE is [0,0,bz])
    V.tensor_scalar(out=cv[:, 0:2], in0=red[:, 6:8], scalar1=1.0 / TM, scalar2=None,
                    op0=OP.mult)
    V.tensor_scalar(out=cv[:, 2], in0=red[:, 8], scalar1=1.0 / TM,
                    scalar2=float(BASE[2]), op0=OP.mult, op1=OP.subtract)

    # suffix cumsum over acts in place: mw becomes w
    for k in range(5, -1, -1):
        V.tensor_tensor(out=mw[:, :, :, k], in0=mw[:, :, :, k], in1=mw[:, :, :, k + 1],
                        op=OP.add)

    # w2 = w - SM∘c folds the former  Htheta = Hphi - c x jsm  stage into the
    # Hphi cross product:  Htheta = D⊙rot + (w - SM∘c) x jacob
    cvb3 = bc(cv[:].unsqueeze(3), (P, 3, J, N_ACT))
    V.tensor_tensor(out=tu[:], in0=SMb, in1=cvb3, op=OP.mult)
    V.tensor_tensor(out=mw[:], in0=mw[:], in1=tu[:], op=OP.subtract)

    # jsm = SM * jacob (reuses delb slot via tag)
    jsm = delb  # overwritten after last delb read (jacob products)
    V.tensor_tensor(out=jsm[:], in0=SMb, in1=jac[:], op=OP.mult)

    # Htheta = D*rot + w2 x jacob  (written into hphi)
    for cx in range(3):
        y, z = (cx + 1) % 3, (cx + 2) % 3
        V.tensor_tensor(out=tu[:, cx], in0=mw[:, y], in1=jac[:, z], op=OP.mult)
        V.tensor_tensor(out=tv[:, cx], in0=mw[:, z], in1=jac[:, y], op=OP.mult)
        V.tensor_tensor(out=hphi[:, cx], in0=tu[:, cx], in1=tv[:, cx], op=OP.subtract)
    DbC = bc(consts[:, 14:35].rearrange("p (c a) -> p c a", c=3).unsqueeze(2),
             (P, 3, CJ, N_ACT))
    for h in range(NCH):
        js = slice(h * CJ, (h + 1) * CJ)
        V.tensor_tensor(out=tu[:, :, js], in0=rots[h], in1=DbC, op=OP.mult)
        V.tensor_tensor(out=hphi[:, :, js], in0=hphi[:, :, js], in1=tu[:, :, js],
                        op=OP.add)

    # cc products and |c|^2, SS
    V.tensor_tensor(out=smalls[:, CC:CC + 3], in0=cv[:], in1=cv[:], op=OP.mult)
    for k, (i, j) in enumerate([(0, 1), (0, 2), (1, 2)]):
        V.tensor_tensor(out=smalls[:, CC + 3 + k], in0=cv[:, i], in1=cv[:, j],
                        op=OP.mult)
    V.tensor_reduce(out=smalls[:, SS_R], in_=red[:, 0:3].transpose([0, 2, 1]),
                    axis=AX.X, op=OP.add)
    V.tensor_reduce(out=smalls[:, CSQ_R], in_=smalls[:, CC:CC + 3].transpose([0, 2, 1]),
                    axis=AX.X, op=OP.add)

    csq_b = bc(smalls[:, CSQ_R].unsqueeze(1), (P, 3, J))
    ss_b = bc(smalls[:, SS_R].unsqueeze(1), (P, 3, J))

    # H_s diag rows HS..HS+2 ; off rows HS+3..HS+5
    a1 = smalls[:, M1_R:M1_R + 2]  # scratch pair rows (reused a lot below)
    V.tensor_tensor(out=smalls[:, T0_R:T0_R + 3], in0=smalls[:, CC:CC + 3], in1=csq_b,
                    op=OP.subtract)
    V.tensor_tensor(out=smalls[:, HS:HS + 3], in0=ss_b, in1=red[:, 0:3], op=OP.subtract)
    nc.vector.scalar_tensor_tensor(out=smalls[:, HS:HS + 3], in0=smalls[:, T0_R:T0_R + 3],
                                   scalar=TM, in1=smalls[:, HS:HS + 3],
                                   op0=OP.mult, op1=OP.add)
    V.tensor_tensor(out=smalls[:, HS:HS + 3], in0=smalls[:, HS:HS + 3], in1=CDb,
                    op=OP.add)
    nc.vector.scalar_tensor_tensor(out=smalls[:, HS + 3:HS + 6],
                                   in0=smalls[:, CC + 3:CC + 6], scalar=TM,
                                   in1=red[:, 3:6], op0=OP.mult, op1=OP.subtract)

    # adjugate (batched pairs via reversed/broadcast row views)
    h = lambda i: smalls[:, HS + i]
    hpair = lambda a, b: smalls[:, HS + a: (HS + b - 1 if b < a else HS + b + 1): (1 if b > a else -1)]
    b2 = lambda ap: bc(ap.unsqueeze(1), (P, 2, J))
    # a11 = h1 h2 - h5^2 ; a22 = h0 h2 - h4^2
    V.tensor_tensor(out=smalls[:, M1_R:M1_R + 2], in0=hpair(1, 0), in1=b2(h(2)), op=OP.mult)
    V.tensor_tensor(out=smalls[:, M2_R:M2_R + 2], in0=hpair(5, 4), in1=hpair(5, 4), op=OP.mult)
    V.tensor_tensor(out=smalls[:, ADJ:ADJ + 2], in0=smalls[:, M1_R:M1_R + 2],
                    in1=smalls[:, M2_R:M2_R + 2], op=OP.subtract)
    # a33 = h0 h1 - h3^2
    V.tensor_tensor(out=smalls[:, T0_R], in0=h(0), in1=h(1), op=OP.mult)
    V.tensor_tensor(out=smalls[:, T1_R], in0=h(3), in1=h(3), op=OP.mult)
    V.tensor_tensor(out=smalls[:, ADJ + 2], in0=smalls[:, T0_R], in1=smalls[:, T1_R],
                    op=OP.subtract)
    # a12 = h4 h5 - h3 h2 ; a13 = h3 h5 - h4 h1
    V.tensor_tensor(out=smalls[:, M1_R:M1_R + 2], in0=hpair(4, 3), in1=b2(h(5)), op=OP.mult)
    V.tensor_tensor(out=smalls[:, M2_R:M2_R + 2], in0=hpair(3, 4), in1=hpair(2, 1), op=OP.mult)
    V.tensor_tensor(out=smalls[:, ADJ + 3:ADJ + 5], in0=smalls[:, M1_R:M1_R + 2],
                    in1=smalls[:, M2_R:M2_R + 2], op=OP.subtract)
    # a23 = h3 h4 - h0 h5
    V.tensor_tensor(out=smalls[:, T0_R], in0=h(3), in1=h(4), op=OP.mult)
    V.tensor_tensor(out=smalls[:, T1_R], in0=h(0), in1=h(5), op=OP.mult)
    V.tensor_tensor(out=smalls[:, ADJ + 5], in0=smalls[:, T0_R], in1=smalls[:, T1_R],
                    op=OP.subtract)

    # det = h0 a11 + h3 a12 + h4 a13 ; A = adj * (-1/det)
    V.tensor_tensor(out=smalls[:, T0_R], in0=h(0), in1=smalls[:, ADJ], op=OP.mult)
    V.tensor_tensor(out=smalls[:, T1_R], in0=h(3), in1=smalls[:, ADJ + 3], op=OP.mult)
    V.tensor_tensor(out=smalls[:, T2_R], in0=h(4), in1=smalls[:, ADJ + 4], op=OP.mult)
    V.tensor_tensor(out=smalls[:, DET_R], in0=smalls[:, T0_R], in1=smalls[:, T1_R],
                    op=OP.add)
    V.tensor_tensor(out=smalls[:, DET_R], in0=smalls[:, DET_R], in1=smalls[:, T2_R],
                    op=OP.add)
    V.reciprocal(out=smalls[:, RDN_R], in_=smalls[:, DET_R])
    rdn_b = bc(smalls[:, RDN_R].unsqueeze(1), (P, 6, J))
    nc.vector.scalar_tensor_tensor(out=smalls[:, ADJ:ADJ + 6],
                                   in0=smalls[:, ADJ:ADJ + 6], scalar=-1.0,
                                   in1=rdn_b, op0=OP.mult, op1=OP.mult)

    # bot = A @ Htheta -> outb cols 21..41 ; top = -(1/TM) jsm + c x bot ->
    # cols 0..20.  Done in two j-halves so the first half's output DMA
    # (168B-contiguous runs) overlaps the second half's compute.
    Arows = [[0, 3, 4], [3, 1, 5], [4, 5, 2]]
    JH = J // 2
    for g in range(2):
        gs = slice(g * JH, (g + 1) * JH)
        cvb = lambda i: bc(cv[:, i, gs].unsqueeze(2), (P, JH, N_ACT))
        Ab = lambda r: bc(smalls[:, ADJ + r, gs].unsqueeze(2), (P, JH, N_ACT))
        bot = lambda c: outb[:, gs, 21 + 7 * c: 28 + 7 * c]
        for oc in range(3):
            r0, r1, r2 = Arows[oc]
            V.tensor_tensor(out=tu[:, 0, gs], in0=Ab(r0), in1=hphi[:, 0, gs], op=OP.mult)
            V.tensor_tensor(out=tu[:, 1, gs], in0=Ab(r1), in1=hphi[:, 1, gs], op=OP.mult)
            V.tensor_tensor(out=tu[:, 2, gs], in0=tu[:, 0, gs], in1=tu[:, 1, gs], op=OP.add)
            V.tensor_tensor(out=tu[:, 0, gs], in0=Ab(r2), in1=hphi[:, 2, gs], op=OP.mult)
            V.tensor_tensor(out=bot(oc), in0=tu[:, 2, gs], in1=tu[:, 0, gs], op=OP.add)
        for cx in range(3):
            y, z = (cx + 1) % 3, (cx + 2) % 3
            V.tensor_tensor(out=tu[:, cx, gs], in0=cvb(y), in1=bot(z), op=OP.mult)
            nc.vector.scalar_tensor_tensor(out=tv[:, cx, gs], in0=jsm[:, cx, gs],
                                           scalar=-1.0 / TM, in1=tu[:, cx, gs],
                                           op0=OP.mult, op1=OP.add)
            V.tensor_tensor(out=tu[:, cx, gs], in0=cvb(z), in1=bot(y), op=OP.mult)
            V.tensor_tensor(out=outb[:, gs, 7 * cx: 7 * cx + 7], in0=tv[:, cx, gs],
                            in1=tu[:, cx, gs], op=OP.subtract)
        nc.sync.dma_start(out=dram["out"][:, gs], in_=outb[:, gs])


@functools.lru_cache(maxsize=1)
def _program():
    from contextlib import ExitStack
    import concourse.bacc as bacc
    import concourse.tile as tile
    from concourse import mybir

    f32 = mybir.dt.float32
    nc = bacc.Bacc("TRN2", target_bir_lowering=False, debug=False)
    dram = {
        "com": nc.dram_tensor("com", [P, J, 21], f32, kind="ExternalInput"),
        "pose": nc.dram_tensor("pose", [P, J, 144], f32, kind="ExternalInput"),
        "consts": nc.dram_tensor("consts", [P, NCONST], f32, kind="ExternalInput"),
        "out": nc.dram_tensor("out", [P, J, 42], f32, kind="ExternalOutput"),
    }
    with tile.TileContext(nc) as tc:
        with ExitStack() as ctx:
            _emit(nc, tc, ctx, dram)
    nc.compile()
    return nc


def _kernel_bm0(com, pose):
    # bm=0 path (not exercised by the shipped setup_inputs; numpy fallback)
    rot = pose[:, :, :3, 2, :N_ACT].copy()
    rot[..., 1] = pose[:, :, :3, 0, 1]
    rot[..., 5] = pose[:, :, :3, 0, 5]
    rot[..., 4] *= -1.0
    delp = pose[:, :, :3, 3, -2][..., None] - pose[:, :, :3, 3, :N_ACT]
    jt = np.cross(rot, delp, axis=2)
    return np.concatenate([jt, rot], axis=2).astype(np.float32)


def kernel(com_list, link_pose_list, bm):
    com_list = np.ascontiguousarray(com_list, dtype=np.float32)
    link_pose_list = np.ascontiguousarray(link_pose_list, dtype=np.float32)
    if not int(bm):
        return _kernel_bm0(com_list, link_pose_list)

    from concourse.bass_utils import run_bass_kernel_spmd

    nc = _program()
    com_flat = com_list.reshape(N_CORES, P, J, 21)
    pose_flat = link_pose_list.reshape(N_CORES, P, J, 144)
    consts = np.broadcast_to(CONSTS, (P, NCONST)).copy()
    in_maps = [
        {"com": com_flat[k], "pose": pose_flat[k], "consts": consts}
        for k in range(N_CORES)
    ]
    res = run_bass_kernel_spmd(nc, in_maps, core_ids=list(range(N_CORES)))
    out = np.stack([res.results[k]["out"] for k in range(N_CORES)])
    return out.reshape(512, 256, 6, 7)



# revision 6
# speedup vs baseline: 1.6596x; 1.6596x over previous
"""Trainium2 Bass kernel for nn_CanadarmJacob (space-arm Jacobian, bm=1 path).

Contract: kernel(**inputs) takes FULL inputs (com_list (512,256,3,7) f32,
link_pose_list (512,256,4,4,9) f32, bm scalar) and returns the FULL output
(512,256,6,7) f32. Internally shards samples across 8 NeuronCores (pure data
parallel), runs a Bass/Tile kernel per core, and gathers.

v2 design: bf16 streams, act-major layout (P, comp, act, J) with J contiguous
so every big op hits the DVE 2x bf16 mode (0.52 ns/elem). Host packs only the
needed pose slices (rot gather + pos) -> 4.4x less input DMA. 3x3 smalls chain
stays f32. Activation engine carries the affine/copy side-channel (sign flip,
row duplication for shifted cross-product views, dtype casts, A-matrix spread).

Math (reformulated from the reference):
  rot   = pose[:3, AXIS[a], a], AXIS=[2,0,2,2,2,0,2]; rot[:,4] *= -1
  del   = com - pos ;  mdel = M del ; mcom = M com
  u     = {mdel_i del_j} (6) ; S = sum_a u ; scom = sum_a mcom (pairwise trees)
  w     = suffix-cumsum_a(mdel) ; jac = rot x del
  c     = scom/TM - BASE ; w2 = w - SM (x) c
  Hth   = D_suf . rot + w2 x jac ; jsm = SM jac
  H_s   = TM(cc^T - |c|^2 I) + diag(CD) + (trS) I - S   (3x3 symmetric)
  A     = -inv(H_s) via adjugate ; bot = A @ Hth ; top = -jsm/TM + c x bot
"""
import sys
import functools

if "/opt/trn_rl_repo" not in sys.path:
    sys.path.insert(0, "/opt/trn_rl_repo")

import numpy as np
import ml_dtypes

BF = ml_dtypes.bfloat16

# ---------------------------------------------------------------- constants
N_CORES = 8
P = 128          # SBUF partitions
J = 128          # samples per partition per core
A = 7            # actuated links
N_ACT = 7

AXIS = np.array([2, 0, 2, 2, 2, 0, 2])
MASS = np.array([105.98, 105.98, 314.98, 279.2, 105.98, 105.98, 243.66], np.float64)
TM = float(MASS.sum() + 100000.0 + 243.66)
DIAGS = np.array([[12.19, 12.19, 3.061], [12.19, 12.19, 3.061], [15.41, 2094.71, 2103.19],
                  [9.522, 1966.28, 1966.28], [8.305, 3.061, 8.0386], [12.13, 12.13, 3.061],
                  [9.336, 44.41, 44.41]], np.float64)
D_SUF = np.cumsum(DIAGS[::-1], axis=0)[::-1]          # (7,3) suffix inertia diag
SM = np.cumsum(MASS[::-1])[::-1]                      # (7,) suffix mass
CD = DIAGS.sum(axis=0)                                # (3,)
_TF0 = np.array([[1, 0, 0, 0], [0, -1, 0, 0], [0, 0, 1.3, 6], [0, 0, 0, 1]], np.float64)
_COM0 = np.array([[1, 0, 0, 0], [0, 1, 0, 0], [0, 0, 1, 0.5], [0, 0, 0, 1]], np.float64)
BASE = (_TF0 @ _COM0)[:3, 3] * 243.66 / (100000.0 + 243.66)   # [0, 0, ~0.0162]

# ctile rows (bf16, each (A, J) broadcast over J): 0=M, 1=SM, 2..4=D_suf[c]
CT = np.broadcast_to(
    np.concatenate([MASS[None, :], SM[None, :], D_SUF.T]).astype(np.float32)[:, :, None],
    (5, A, J)).astype(BF)
# f32 per-partition consts row: CD (3)
CONSTS32 = np.array(list(CD) + [float(BASE[2])], np.float32)
NC32 = CONSTS32.shape[0]


def _emit(nc, tc, ctx, dram):
    from concourse import mybir

    f32 = mybir.dt.float32
    b16 = mybir.dt.bfloat16
    OP = mybir.AluOpType
    V = nc.vector
    SE = nc.scalar           # Activation engine
    Copy = mybir.ActivationFunctionType.Copy

    pool = ctx.enter_context(tc.tile_pool(name="main", bufs=1))

    # ---- tiles (act-major: last dim J contiguous) -------------------------
    ctile = pool.tile([P, 5, A, J], b16)       # M, SM, Dx, Dy, Dz
    c32 = pool.tile([P, NC32], f32)
    rot5 = pool.tile([P, 5, A, J], b16)        # rows 0-2 rot, 3-4 dup(x,y)
    pos = pool.tile([P, 3, A, J], b16)
    com = pool.tile([P, 3, A, J], b16)
    del5 = pool.tile([P, 5, A, J], b16)
    mdel = pool.tile([P, 3, A, J], b16)        # becomes w in place (suffix cumsum)
    prods = pool.tile([P, 9, A, J], b16)       # u rows 0-5 [xx,yy,zz,xy,yz,xz], mcom 6-8
    tl1 = pool.tile([P, 9, 3, J], b16)         # tree L1
    tc0 = pool.tile([P, 9, J], b16)            # tree L2 left
    tc1 = pool.tile([P, 9, J], b16)            # tree L2 right
    sums = pool.tile([P, 9, J], b16)           # S rows 0-5, scom rows 6-8
    jac5 = pool.tile([P, 5, A, J], b16)
    scr1 = pool.tile([P, 3, A, J], b16)
    scr2 = pool.tile([P, 3, A, J], b16)
    smc = pool.tile([P, 3, A, J], b16)
    w25 = pool.tile([P, 5, A, J], b16)
    hth = pool.tile([P, 3, A, J], b16)
    jsm = pool.tile([P, 3, A, J], b16)
    m9 = pool.tile([P, 9, A, J], b16)
    bot5 = pool.tile([P, 5, A, J], b16)
    top3 = pool.tile([P, 3, A, J], b16)
    c5 = pool.tile([P, 5, J], f32)             # c rows x,y,z,x,y (f32)
    cb5 = pool.tile([P, 5, J], b16)            # c in bf16 + dup
    sm = pool.tile([P, 26, J], f32)            # smalls scratch
    abf = pool.tile([P, 6, J], b16)            # A upper-tri [00,01,02,11,12,22]
    a9 = pool.tile([P, 9, J], b16)             # A row-major 3x3

    # smalls row map (sm tile)
    CC = 0      # rows 0-2 diag(xx,yy,zz), 3-5 off (xy,yz,xz)
    CSQ = 6
    SSR = 7
    T3 = 8      # rows 8-10
    D3 = 11     # rows 11-13
    HS = 14     # rows 14-19: [h00,h11,h22,h01,h12,h02]
    ADJ = 20    # rows 20-25 order [A00,A01,A02,A11,A12,A22]
    MA = 8      # scratch pair rows 8-9 (T3 dead after HS built)
    MB = 10     # scratch pair rows 10-11
    DET = 12
    RDET = 13

    # ---- input DMAs -------------------------------------------------------
    nc.sync.dma_start(out=ctile[:, 0:1], in_=dram["ctm"][:])     # M row first
    nc.sync.dma_start(out=pos[:], in_=dram["pos"][:])
    nc.sync.dma_start(out=com[:], in_=dram["com"][:])
    nc.sync.dma_start(out=rot5[:, 0:3], in_=dram["rot"][:])
    nc.sync.dma_start(out=ctile[:, 1:5], in_=dram["ctr"][:])
    nc.sync.dma_start(out=c32[:], in_=dram["c32"][:])

    Mb = ctile[:, 0].unsqueeze(1).broadcast_to((P, 3, A, J))
    SMb = ctile[:, 1].unsqueeze(1).broadcast_to((P, 3, A, J))
    Dt = ctile[:, 2:5]

    # ---- streams (DVE unless noted) --------------------------------------
    # sign flip rot act 4 (Act engine), then dup rows for shifted views
    SE.mul(rot5[:, 0:3, 4], rot5[:, 0:3, 4], -1.0)
    SE.copy(rot5[:, 3:5], rot5[:, 0:2])

    V.tensor_tensor(out=del5[:, 0:3], in0=com[:], in1=pos[:], op=OP.subtract)
    SE.copy(del5[:, 3:5], del5[:, 0:2])
    V.tensor_tensor(out=mdel[:], in0=Mb, in1=del5[:, 0:3], op=OP.mult)
    V.tensor_tensor(out=prods[:, 6:9], in0=Mb, in1=com[:], op=OP.mult)

    # u products: diag (xx,yy,zz), off2 (xy,yz), off1 (xz)
    V.tensor_tensor(out=prods[:, 0:3], in0=mdel[:], in1=del5[:, 0:3], op=OP.mult)
    V.tensor_tensor(out=prods[:, 3:5], in0=mdel[:, 0:2], in1=del5[:, 1:3], op=OP.mult)
    V.tensor_tensor(out=prods[:, 5], in0=mdel[:, 0], in1=del5[:, 2], op=OP.mult)

    # pairwise act-sum tree over prods: (7) -> S rows 0-5, scom rows 6-8
    V.tensor_tensor(out=tl1[:], in0=prods[:, :, 0:3], in1=prods[:, :, 4:7], op=OP.add)
    V.tensor_tensor(out=tc0[:], in0=tl1[:, :, 0], in1=tl1[:, :, 1], op=OP.add)
    V.tensor_tensor(out=tc1[:], in0=tl1[:, :, 2], in1=prods[:, :, 3], op=OP.add)
    V.tensor_tensor(out=sums[:], in0=tc0[:], in1=tc1[:], op=OP.add)

    # jac = rot x del via shifted dup views
    V.tensor_tensor(out=scr1[:], in0=rot5[:, 1:4], in1=del5[:, 2:5], op=OP.mult)
    V.tensor_tensor(out=scr2[:], in0=rot5[:, 2:5], in1=del5[:, 1:4], op=OP.mult)
    V.tensor_tensor(out=jac5[:, 0:3], in0=scr1[:], in1=scr2[:], op=OP.subtract)
    SE.copy(jac5[:, 3:5], jac5[:, 0:2])

    # w: suffix cumsum over acts, in place in mdel
    for k in range(A - 2, -1, -1):
        V.tensor_tensor(out=mdel[:, :, k], in0=mdel[:, :, k], in1=mdel[:, :, k + 1],
                        op=OP.add)

    # c = scom/TM - BASE (Act engine), then bf16 copy + dups
    SE.mul(c5[:, 0:2], sums[:, 6:8], 1.0 / TM)
    SE.activation(c5[:, 2], sums[:, 8], Copy, bias=-float(BASE[2]), scale=1.0 / TM)
    SE.copy(c5[:, 3:5], c5[:, 0:2])
    SE.copy(cb5[:, 0:3], c5[:, 0:3])
    SE.copy(cb5[:, 3:5], cb5[:, 0:2])

    # w2 = w - SM (x) c
    cbb = cb5[:, 0:3].unsqueeze(2).broadcast_to((P, 3, A, J))
    V.tensor_tensor(out=smc[:], in0=SMb, in1=cbb, op=OP.mult)
    V.tensor_tensor(out=w25[:, 0:3], in0=mdel[:], in1=smc[:], op=OP.subtract)
    SE.copy(w25[:, 3:5], w25[:, 0:2])

    # jsm = SM jac
    V.tensor_tensor(out=jsm[:], in0=SMb, in1=jac5[:, 0:3], op=OP.mult)

    # Hth = D.rot + w2 x jac
    V.tensor_tensor(out=smc[:], in0=Dt, in1=rot5[:, 0:3], op=OP.mult)
    V.tensor_tensor(out=scr1[:], in0=w25[:, 1:4], in1=jac5[:, 2:5], op=OP.mult)
    V.tensor_tensor(out=scr2[:], in0=w25[:, 2:5], in1=jac5[:, 1:4], op=OP.mult)
    V.tensor_tensor(out=scr1[:], in0=scr1[:], in1=scr2[:], op=OP.subtract)
    V.tensor_tensor(out=hth[:], in0=smc[:], in1=scr1[:], op=OP.add)

    # ---- smalls (f32) ----------------------------------------------------
    # cc products via dup'd c5: diag then off [xy,yz,xz]
    V.tensor_tensor(out=sm[:, CC:CC + 3], in0=c5[:, 0:3], in1=c5[:, 0:3], op=OP.mult)
    V.tensor_tensor(out=sm[:, CC + 3:CC + 6], in0=c5[:, 0:3], in1=c5[:, 1:4], op=OP.mult)
    V.tensor_tensor(out=sm[:, CSQ], in0=sm[:, CC], in1=sm[:, CC + 1], op=OP.add)
    V.tensor_tensor(out=sm[:, CSQ], in0=sm[:, CSQ], in1=sm[:, CC + 2], op=OP.add)
    V.tensor_tensor(out=sm[:, SSR], in0=sums[:, 0], in1=sums[:, 1], op=OP.add)
    V.tensor_tensor(out=sm[:, SSR], in0=sm[:, SSR], in1=sums[:, 2], op=OP.add)

    CDb = c32[:, 0:3].unsqueeze(2).broadcast_to((P, 3, J))
    ssb = sm[:, SSR].unsqueeze(1).broadcast_to((P, 3, J))
    csqb = sm[:, CSQ].unsqueeze(1).broadcast_to((P, 3, J))
    V.tensor_tensor(out=sm[:, T3:T3 + 3], in0=CDb, in1=sums[:, 0:3], op=OP.subtract)
    V.tensor_tensor(out=sm[:, T3:T3 + 3], in0=sm[:, T3:T3 + 3], in1=ssb, op=OP.add)
    V.tensor_tensor(out=sm[:, D3:D3 + 3], in0=sm[:, CC:CC + 3], in1=csqb, op=OP.subtract)
    V.scalar_tensor_tensor(out=sm[:, HS:HS + 3], in0=sm[:, D3:D3 + 3], scalar=TM,
                           in1=sm[:, T3:T3 + 3], op0=OP.mult, op1=OP.add)
    V.scalar_tensor_tensor(out=sm[:, HS + 3:HS + 6], in0=sm[:, CC + 3:CC + 6],
                           scalar=TM, in1=sums[:, 3:6], op0=OP.mult, op1=OP.subtract)

    # adjugate of Hs rows [h00,h11,h22,h01,h12,h02] -> adj [A00,A01,A02,A11,A12,A22]
    h = lambda i: sm[:, HS + i]
    m2a = sm[:, MA:MA + 2]
    m2b = sm[:, MB:MB + 2]
    # A00 = h11 h22 - h12^2 ; A11 = h00 h22 - h02^2
    V.tensor_tensor(out=m2a, in0=sm[:, HS + 1:HS - 1:-1],
                    in1=h(2).unsqueeze(1).broadcast_to((P, 2, J)), op=OP.mult)
    V.tensor_tensor(out=m2b, in0=sm[:, HS + 4:HS + 6],
                    in1=sm[:, HS + 4:HS + 6], op=OP.mult)
    V.tensor_tensor(out=sm[:, ADJ:ADJ + 4:3], in0=m2a, in1=m2b, op=OP.subtract)
    # A22 = h00 h11 - h01^2 ; A02 = h01 h12 - h02 h11
    V.tensor_tensor(out=m2a, in0=sm[:, HS:HS + 4:3], in1=sm[:, HS + 1:HS + 5:3],
                    op=OP.mult)
    V.tensor_tensor(out=m2b, in0=sm[:, HS + 3:HS + 6:2], in1=sm[:, HS + 3:HS - 1:-2],
                    op=OP.mult)
    V.tensor_tensor(out=sm[:, ADJ + 5:ADJ + 1:-3], in0=m2a, in1=m2b, op=OP.subtract)
    # A01 = h02 h12 - h01 h22
    V.tensor_tensor(out=sm[:, MA], in0=h(5), in1=h(4), op=OP.mult)
    V.tensor_tensor(out=sm[:, MA + 1], in0=h(3), in1=h(2), op=OP.mult)
    V.tensor_tensor(out=sm[:, ADJ + 1], in0=sm[:, MA], in1=sm[:, MA + 1], op=OP.subtract)
    # A12 = h01 h02 - h12 h00
    V.tensor_tensor(out=sm[:, MB], in0=h(3), in1=h(5), op=OP.mult)
    V.tensor_tensor(out=sm[:, MB + 1], in0=h(4), in1=h(0), op=OP.mult)
    V.tensor_tensor(out=sm[:, ADJ + 4], in0=sm[:, MB], in1=sm[:, MB + 1], op=OP.subtract)

    # det = h00 A00 + h01 A01 + h02 A02 ; A(bf16) = adj * (-1/det)
    V.tensor_tensor(out=sm[:, DET], in0=h(0), in1=sm[:, ADJ], op=OP.mult)
    V.tensor_tensor(out=sm[:, MA], in0=h(3), in1=sm[:, ADJ + 1], op=OP.mult)
    V.tensor_tensor(out=sm[:, DET], in0=sm[:, DET], in1=sm[:, MA], op=OP.add)
    V.tensor_tensor(out=sm[:, MB], in0=h(5), in1=sm[:, ADJ + 2], op=OP.mult)
    V.tensor_tensor(out=sm[:, DET], in0=sm[:, DET], in1=sm[:, MB], op=OP.add)
    V.reciprocal(out=sm[:, RDET], in_=sm[:, DET])
    V.scalar_tensor_tensor(out=abf[:], in0=sm[:, ADJ:ADJ + 6], scalar=-1.0,
                           in1=sm[:, RDET].unsqueeze(1).broadcast_to((P, 6, J)),
                           op0=OP.mult, op1=OP.mult)

    # spread A upper-tri -> row-major 3x3 (Act engine copies)
    SE.copy(a9[:, 0:3], abf[:, 0:3])
    SE.copy(a9[:, 3], abf[:, 1])
    SE.copy(a9[:, 4:6], abf[:, 3:5])
    SE.copy(a9[:, 6], abf[:, 2])
    SE.copy(a9[:, 7:9], abf[:, 4:6])

    # ---- bot = A @ Hth : m9[3r+c] = A[r,c]*Hth[c], then sum over c --------
    m9v = m9[:].rearrange("p (r c) a j -> p r c a j", r=3)
    a9v = a9[:].rearrange("p (r c) j -> p r c j", r=3).unsqueeze(3) \
        .broadcast_to((P, 3, 3, A, J))
    hthv = hth[:].unsqueeze(1).broadcast_to((P, 3, 3, A, J))
    V.tensor_tensor(out=m9v, in0=a9v, in1=hthv, op=OP.mult)
    V.tensor_tensor(out=bot5[:, 0:3], in0=m9[:, 0:9:3], in1=m9[:, 1:9:3], op=OP.add)
    V.tensor_tensor(out=bot5[:, 0:3], in0=bot5[:, 0:3], in1=m9[:, 2:9:3], op=OP.add)
    SE.copy(bot5[:, 3:5], bot5[:, 0:2])

    # ---- top = -jsm/TM + c x bot -----------------------------------------
    cbb1 = cb5[:, 1:4].unsqueeze(2).broadcast_to((P, 3, A, J))
    cbb2 = cb5[:, 2:5].unsqueeze(2).broadcast_to((P, 3, A, J))
    V.tensor_tensor(out=scr1[:], in0=cbb1, in1=bot5[:, 2:5], op=OP.mult)
    V.tensor_tensor(out=scr2[:], in0=cbb2, in1=bot5[:, 1:4], op=OP.mult)
    V.tensor_tensor(out=scr1[:], in0=scr1[:], in1=scr2[:], op=OP.subtract)
    V.tensor_scalar_mul(scr2[:], jsm[:], -1.0 / TM)
    V.tensor_tensor(out=top3[:], in0=scr1[:], in1=scr2[:], op=OP.add)

    # ---- output DMAs ------------------------------------------------------
    nc.sync.dma_start(out=dram["out"][:, 0:3], in_=top3[:])
    nc.sync.dma_start(out=dram["out"][:, 3:6], in_=bot5[:, 0:3])


@functools.lru_cache(maxsize=1)
def _program():
    from contextlib import ExitStack
    import concourse.bacc as bacc
    import concourse.tile as tile
    from concourse import mybir

    f32 = mybir.dt.float32
    b16 = mybir.dt.bfloat16
    nc = bacc.Bacc("TRN2", target_bir_lowering=False, debug=False)
    dram = {
        "rot": nc.dram_tensor("rot", [P, 3, A, J], b16, kind="ExternalInput"),
        "pos": nc.dram_tensor("pos", [P, 3, A, J], b16, kind="ExternalInput"),
        "com": nc.dram_tensor("com", [P, 3, A, J], b16, kind="ExternalInput"),
        "ctm": nc.dram_tensor("ctm", [P, 1, A, J], b16, kind="ExternalInput"),
        "ctr": nc.dram_tensor("ctr", [P, 4, A, J], b16, kind="ExternalInput"),
        "c32": nc.dram_tensor("c32", [P, NC32], f32, kind="ExternalInput"),
        "out": nc.dram_tensor("out", [P, 6, A, J], b16, kind="ExternalOutput"),
    }
    with tile.TileContext(nc) as tc:
        with ExitStack() as ctx:
            _emit(nc, tc, ctx, dram)
    nc.compile()
    return nc


def pack_inputs(com_list, link_pose_list):
    """Host-side layout packing (pure data movement + dtype cast)."""
    N = N_CORES * P * J
    pose = np.ascontiguousarray(link_pose_list, dtype=np.float32).reshape(N, 4, 4, 9)
    com = np.ascontiguousarray(com_list, dtype=np.float32).reshape(N, 3, 7)
    rot = pose[:, :3, AXIS, np.arange(7)]                # (N, 3, 7)
    pos = pose[:, :3, 3, :7]                             # (N, 3, 7)

    def to_core_layout(x):  # (N, 3, 7) -> (cores, P, 3, 7, J)
        return np.ascontiguousarray(
            x.reshape(N_CORES, P, J, 3, 7).transpose(0, 1, 3, 4, 2)).astype(BF)

    return to_core_layout(rot), to_core_layout(pos), to_core_layout(com)


def make_in_maps(rot, pos, com):
    ctm = np.ascontiguousarray(np.broadcast_to(CT[0:1], (P, 1, A, J)))
    ctr = np.ascontiguousarray(np.broadcast_to(CT[1:5], (P, 4, A, J)))
    c32 = np.broadcast_to(CONSTS32, (P, NC32)).copy()
    return [
        {"rot": rot[k], "pos": pos[k], "com": com[k], "ctm": ctm, "ctr": ctr,
         "c32": c32}
        for k in range(N_CORES)
    ]


def unpack_output(res):
    out = np.stack([res.results[k]["out"] for k in range(N_CORES)])  # (8,P,6,7,J) bf16
    out = out.astype(np.float32).transpose(0, 1, 4, 2, 3)            # (8,P,J,6,7)
    return np.ascontiguousarray(out.reshape(512, 256, 6, 7))


def _kernel_bm0(com, pose):
    # bm=0 path (not exercised by the shipped setup_inputs; numpy fallback)
    rot = pose[:, :, :3, 2, :N_ACT].copy()
    rot[..., 1] = pose[:, :, :3, 0, 1]
    rot[..., 5] = pose[:, :, :3, 0, 5]
    rot[..., 4] *= -1.0
    delp = pose[:, :, :3, 3, -2][..., None] - pose[:, :, :3, 3, :N_ACT]
    jt = np.cross(rot, delp, axis=2)
    return np.concatenate([jt, rot], axis=2).astype(np.float32)


def kernel(com_list, link_pose_list, bm):
    if not int(bm):
        return _kernel_bm0(np.asarray(com_list, np.float32),
                           np.asarray(link_pose_list, np.float32))

    from concourse.bass_utils import run_bass_kernel_spmd

    nc = _program()
    rot, pos, com = pack_inputs(com_list, link_pose_list)
    res = run_bass_kernel_spmd(nc, make_in_maps(rot, pos, com),
                               core_ids=list(range(N_CORES)))
    return unpack_output(res)


# revision 15
# speedup vs baseline: 1.6702x; 1.0064x over previous
"""Trainium2 Bass kernel for nn_CanadarmJacob (space-arm Jacobian, bm=1 path).

Contract: kernel(**inputs) takes FULL inputs (com_list (512,256,3,7) f32,
link_pose_list (512,256,4,4,9) f32, bm scalar) and returns the FULL output
(512,256,6,7) f32. Internally shards samples across 8 NeuronCores (pure data
parallel), runs a Bass/Tile kernel per core, and gathers.

v2 design: bf16 streams, act-major layout (P, comp, act, J) with J contiguous
so every big op hits the DVE 2x bf16 mode (0.52 ns/elem). Host packs only the
needed pose slices (rot gather + pos) -> 4.4x less input DMA. 3x3 smalls chain
stays f32. Activation engine carries the affine/copy side-channel (sign flip,
row duplication for shifted cross-product views, dtype casts, A-matrix spread).

Math (reformulated from the reference):
  rot   = pose[:3, AXIS[a], a], AXIS=[2,0,2,2,2,0,2]; rot[:,4] *= -1
  del   = com - pos ;  mdel = M del ; mcom = M com
  u     = {mdel_i del_j} (6) ; S = sum_a u ; scom = sum_a mcom (pairwise trees)
  w     = suffix-cumsum_a(mdel) ; jac = rot x del
  c     = scom/TM - BASE ; w2 = w - SM (x) c
  Hth   = D_suf . rot + w2 x jac ; jsm = SM jac
  H_s   = TM(cc^T - |c|^2 I) + diag(CD) + (trS) I - S   (3x3 symmetric)
  A     = -inv(H_s) via adjugate ; bot = A @ Hth ; top = -jsm/TM + c x bot
"""
import sys
import functools

if "/opt/trn_rl_repo" not in sys.path:
    sys.path.insert(0, "/opt/trn_rl_repo")

import numpy as np
import ml_dtypes

BF = ml_dtypes.bfloat16

# ---------------------------------------------------------------- constants
N_CORES = 8
P = 128          # SBUF partitions
J = 128          # samples per partition per core
A = 7            # actuated links
N_ACT = 7

AXIS = np.array([2, 0, 2, 2, 2, 0, 2])
MASS = np.array([105.98, 105.98, 314.98, 279.2, 105.98, 105.98, 243.66], np.float64)
TM = float(MASS.sum() + 100000.0 + 243.66)
DIAGS = np.array([[12.19, 12.19, 3.061], [12.19, 12.19, 3.061], [15.41, 2094.71, 2103.19],
                  [9.522, 1966.28, 1966.28], [8.305, 3.061, 8.0386], [12.13, 12.13, 3.061],
                  [9.336, 44.41, 44.41]], np.float64)
D_SUF = np.cumsum(DIAGS[::-1], axis=0)[::-1]          # (7,3) suffix inertia diag
SM = np.cumsum(MASS[::-1])[::-1]                      # (7,) suffix mass
CD = DIAGS.sum(axis=0)                                # (3,)
_TF0 = np.array([[1, 0, 0, 0], [0, -1, 0, 0], [0, 0, 1.3, 6], [0, 0, 0, 1]], np.float64)
_COM0 = np.array([[1, 0, 0, 0], [0, 1, 0, 0], [0, 0, 1, 0.5], [0, 0, 0, 1]], np.float64)
BASE = (_TF0 @ _COM0)[:3, 3] * 243.66 / (100000.0 + 243.66)   # [0, 0, ~0.0162]

# ctile rows (bf16, each (A, J) broadcast over J):
#   0=M, 1=SM, 2..4=D_suf[c], 5=-SM/TM
CT = np.broadcast_to(
    np.concatenate([MASS[None, :], SM[None, :], D_SUF.T,
                    (-SM / TM)[None, :]]).astype(np.float32)[:, :, None],
    (6, A, J)).astype(BF)
# f32 per-partition consts row: CD (3)
CONSTS32 = np.array(list(CD) + [float(BASE[2])], np.float32)
NC32 = CONSTS32.shape[0]


def _emit(nc, tc, ctx, dram):
    from concourse import mybir

    f32 = mybir.dt.float32
    b16 = mybir.dt.bfloat16
    OP = mybir.AluOpType
    V = nc.vector
    SE = nc.scalar           # Activation engine
    Copy = mybir.ActivationFunctionType.Copy

    pool = ctx.enter_context(tc.tile_pool(name="main", bufs=1))

    # ---- tiles (act-major: last dim J contiguous) -------------------------
    ctile = pool.tile([P, 6, A, J], b16)       # M, SM, Dx, Dy, Dz, -SM/TM
    c32 = pool.tile([P, NC32], f32)
    rot5 = pool.tile([P, 5, A, J], b16)        # rows 0-2 rot, 3-4 dup(x,y)
    pos = pool.tile([P, 3, A, J], b16)
    com = pool.tile([P, 3, A, J], b16)
    del5 = pool.tile([P, 5, A, J], b16)
    mdel = pool.tile([P, 3, A, J], b16)        # becomes w in place (suffix cumsum)
    prods = pool.tile([P, 9, A, J], b16)       # u rows 0-5 [xx,yy,zz,xy,yz,xz], mcom 6-8
    tl1 = pool.tile([P, 9, 3, J], b16)         # tree L1
    tc0 = pool.tile([P, 9, J], b16)            # tree L2 left
    tc1 = pool.tile([P, 9, J], b16)            # tree L2 right
    sums = pool.tile([P, 9, J], b16)           # S rows 0-5, scom rows 6-8
    jac5 = pool.tile([P, 5, A, J], b16)
    scr1 = pool.tile([P, 3, A, J], b16)
    scr2 = pool.tile([P, 3, A, J], b16)
    smc = pool.tile([P, 3, A, J], b16)
    w25 = pool.tile([P, 5, A, J], b16)
    hth = pool.tile([P, 3, A, J], b16)
    m9 = pool.tile([P, 9, A, J], b16)
    bot5 = pool.tile([P, 5, A, J], b16)
    top3 = pool.tile([P, 3, A, J], b16)
    c5 = pool.tile([P, 5, J], f32)             # c rows x,y,z,x,y (f32)
    cb5 = pool.tile([P, 5, J], b16)            # c in bf16 + dup
    sm = pool.tile([P, 26, J], f32)            # smalls scratch
    abf = pool.tile([P, 6, J], b16)            # A upper-tri [00,01,02,11,12,22]
    a9 = pool.tile([P, 9, J], b16)             # A row-major 3x3

    # smalls row map (sm tile)
    CC = 0      # rows 0-2 diag(xx,yy,zz), 3-5 off (xy,yz,xz)
    CSQ = 6
    SSR = 7
    T3 = 8      # rows 8-10
    D3 = 11     # rows 11-13
    HS = 14     # rows 14-19: [h00,h11,h22,h01,h12,h02]
    ADJ = 20    # rows 20-25 order [A00,A01,A02,A11,A12,A22]
    MA = 8      # scratch pair rows 8-9 (T3 dead after HS built)
    MB = 10     # scratch pair rows 10-11
    DET = 12
    RDET = 13

    # ---- input DMAs (halves of pos/com first so compute starts early) ----
    JH = J // 2
    nc.sync.dma_start(out=pos[:, :, :, 0:JH], in_=dram["pos"][:, 0])
    nc.sync.dma_start(out=com[:, :, :, 0:JH], in_=dram["com"][:, 0])
    nc.sync.dma_start(out=ctile[:, 0:1], in_=dram["ctm"][:])     # M row
    nc.sync.dma_start(out=pos[:, :, :, JH:J], in_=dram["pos"][:, 1])
    nc.sync.dma_start(out=com[:, :, :, JH:J], in_=dram["com"][:, 1])
    nc.sync.dma_start(out=rot5[:, 0:3, :, 0:JH], in_=dram["rot"][:, 0])
    nc.sync.dma_start(out=rot5[:, 0:3, :, JH:J], in_=dram["rot"][:, 1])
    nc.sync.dma_start(out=ctile[:, 1:6], in_=dram["ctr"][:])
    nc.sync.dma_start(out=c32[:], in_=dram["c32"][:])

    Mb = ctile[:, 0].unsqueeze(1).broadcast_to((P, 3, A, J))
    SMb = ctile[:, 1].unsqueeze(1).broadcast_to((P, 3, A, J))
    Dt = ctile[:, 2:5]

    # ---- streams (DVE unless noted) --------------------------------------
    # early per-half stage: del, mdel, mcom, u products
    for hjs in (slice(0, JH), slice(JH, J)):
        Mbh = ctile[:, 0, :, hjs].unsqueeze(1).broadcast_to((P, 3, A, JH))
        V.tensor_tensor(out=del5[:, 0:3, :, hjs], in0=com[:, :, :, hjs],
                        in1=pos[:, :, :, hjs], op=OP.subtract)
        V.tensor_tensor(out=mdel[:, :, :, hjs], in0=Mbh,
                        in1=del5[:, 0:3, :, hjs], op=OP.mult)
        V.tensor_tensor(out=prods[:, 6:9, :, hjs], in0=Mbh,
                        in1=com[:, :, :, hjs], op=OP.mult)
        V.tensor_tensor(out=prods[:, 0:3, :, hjs], in0=mdel[:, :, :, hjs],
                        in1=del5[:, 0:3, :, hjs], op=OP.mult)
        V.tensor_tensor(out=prods[:, 3:5, :, hjs], in0=mdel[:, 0:2, :, hjs],
                        in1=del5[:, 1:3, :, hjs], op=OP.mult)
        V.tensor_tensor(out=prods[:, 5, :, hjs], in0=mdel[:, 0, :, hjs],
                        in1=del5[:, 2, :, hjs], op=OP.mult)

    # sign flip rot act 4 (Act engine), then dup rows for shifted views
    SE.mul(rot5[:, 0:3, 4], rot5[:, 0:3, 4], -1.0)
    SE.copy(rot5[:, 3:5], rot5[:, 0:2])
    SE.copy(del5[:, 3:5], del5[:, 0:2])

    # pairwise act-sum tree over prods: (7) -> S rows 0-5, scom rows 6-8
    V.tensor_tensor(out=tl1[:], in0=prods[:, :, 0:3], in1=prods[:, :, 4:7], op=OP.add)
    V.tensor_tensor(out=tc0[:], in0=tl1[:, :, 0], in1=tl1[:, :, 1], op=OP.add)
    V.tensor_tensor(out=tc1[:], in0=tl1[:, :, 2], in1=prods[:, :, 3], op=OP.add)
    V.tensor_tensor(out=sums[:], in0=tc0[:], in1=tc1[:], op=OP.add)

    # jac = rot x del via shifted dup views
    V.tensor_tensor(out=scr1[:], in0=rot5[:, 1:4], in1=del5[:, 2:5], op=OP.mult)
    V.tensor_tensor(out=scr2[:], in0=rot5[:, 2:5], in1=del5[:, 1:4], op=OP.mult)
    V.tensor_tensor(out=jac5[:, 0:3], in0=scr1[:], in1=scr2[:], op=OP.subtract)
    SE.copy(jac5[:, 3:5], jac5[:, 0:2])

    # w: suffix cumsum over acts, in place in mdel
    for k in range(A - 2, -1, -1):
        V.tensor_tensor(out=mdel[:, :, k], in0=mdel[:, :, k], in1=mdel[:, :, k + 1],
                        op=OP.add)

    # c = scom/TM - BASE (Act engine), then bf16 copy + dups
    SE.mul(c5[:, 0:2], sums[:, 6:8], 1.0 / TM)
    SE.activation(c5[:, 2], sums[:, 8], Copy, bias=-float(BASE[2]), scale=1.0 / TM)
    SE.copy(c5[:, 3:5], c5[:, 0:2])
    SE.copy(cb5[:, 0:3], c5[:, 0:3])
    SE.copy(cb5[:, 3:5], cb5[:, 0:2])

    # w2 = w - SM (x) c
    cbb = cb5[:, 0:3].unsqueeze(2).broadcast_to((P, 3, A, J))
    V.tensor_tensor(out=smc[:], in0=SMb, in1=cbb, op=OP.mult)
    V.tensor_tensor(out=w25[:, 0:3], in0=mdel[:], in1=smc[:], op=OP.subtract)
    SE.copy(w25[:, 3:5], w25[:, 0:2])

    # Hth = D.rot + w2 x jac
    V.tensor_tensor(out=smc[:], in0=Dt, in1=rot5[:, 0:3], op=OP.mult)
    V.tensor_tensor(out=scr1[:], in0=w25[:, 1:4], in1=jac5[:, 2:5], op=OP.mult)
    V.tensor_tensor(out=scr2[:], in0=w25[:, 2:5], in1=jac5[:, 1:4], op=OP.mult)
    V.tensor_tensor(out=scr1[:], in0=scr1[:], in1=scr2[:], op=OP.subtract)
    V.tensor_tensor(out=hth[:], in0=smc[:], in1=scr1[:], op=OP.add)

    # ---- smalls (f32) ----------------------------------------------------
    # cc products via dup'd c5: diag then off [xy,yz,xz]
    V.tensor_tensor(out=sm[:, CC:CC + 3], in0=c5[:, 0:3], in1=c5[:, 0:3], op=OP.mult)
    V.tensor_tensor(out=sm[:, CC + 3:CC + 6], in0=c5[:, 0:3], in1=c5[:, 1:4], op=OP.mult)
    V.tensor_tensor(out=sm[:, CSQ], in0=sm[:, CC], in1=sm[:, CC + 1], op=OP.add)
    V.tensor_tensor(out=sm[:, CSQ], in0=sm[:, CSQ], in1=sm[:, CC + 2], op=OP.add)
    V.tensor_tensor(out=sm[:, SSR], in0=sums[:, 0], in1=sums[:, 1], op=OP.add)
    V.tensor_tensor(out=sm[:, SSR], in0=sm[:, SSR], in1=sums[:, 2], op=OP.add)

    CDb = c32[:, 0:3].unsqueeze(2).broadcast_to((P, 3, J))
    ssb = sm[:, SSR].unsqueeze(1).broadcast_to((P, 3, J))
    csqb = sm[:, CSQ].unsqueeze(1).broadcast_to((P, 3, J))
    V.tensor_tensor(out=sm[:, T3:T3 + 3], in0=CDb, in1=sums[:, 0:3], op=OP.subtract)
    V.tensor_tensor(out=sm[:, T3:T3 + 3], in0=sm[:, T3:T3 + 3], in1=ssb, op=OP.add)
    V.tensor_tensor(out=sm[:, D3:D3 + 3], in0=sm[:, CC:CC + 3], in1=csqb, op=OP.subtract)
    V.scalar_tensor_tensor(out=sm[:, HS:HS + 3], in0=sm[:, D3:D3 + 3], scalar=TM,
                           in1=sm[:, T3:T3 + 3], op0=OP.mult, op1=OP.add)
    V.scalar_tensor_tensor(out=sm[:, HS + 3:HS + 6], in0=sm[:, CC + 3:CC + 6],
                           scalar=TM, in1=sums[:, 3:6], op0=OP.mult, op1=OP.subtract)

    # adjugate of Hs rows [h00,h11,h22,h01,h12,h02] -> adj [A00,A01,A02,A11,A12,A22]
    h = lambda i: sm[:, HS + i]
    m2a = sm[:, MA:MA + 2]
    m2b = sm[:, MB:MB + 2]
    # A00 = h11 h22 - h12^2 ; A11 = h00 h22 - h02^2
    V.tensor_tensor(out=m2a, in0=sm[:, HS + 1:HS - 1:-1],
                    in1=h(2).unsqueeze(1).broadcast_to((P, 2, J)), op=OP.mult)
    V.tensor_tensor(out=m2b, in0=sm[:, HS + 4:HS + 6],
                    in1=sm[:, HS + 4:HS + 6], op=OP.mult)
    V.tensor_tensor(out=sm[:, ADJ:ADJ + 4:3], in0=m2a, in1=m2b, op=OP.subtract)
    # A22 = h00 h11 - h01^2 ; A02 = h01 h12 - h02 h11
    V.tensor_tensor(out=m2a, in0=sm[:, HS:HS + 4:3], in1=sm[:, HS + 1:HS + 5:3],
                    op=OP.mult)
    V.tensor_tensor(out=m2b, in0=sm[:, HS + 3:HS + 6:2], in1=sm[:, HS + 3:HS - 1:-2],
                    op=OP.mult)
    V.tensor_tensor(out=sm[:, ADJ + 5:ADJ + 1:-3], in0=m2a, in1=m2b, op=OP.subtract)
    # A01 = h02 h12 - h01 h22
    V.tensor_tensor(out=sm[:, MA], in0=h(5), in1=h(4), op=OP.mult)
    V.tensor_tensor(out=sm[:, MA + 1], in0=h(3), in1=h(2), op=OP.mult)
    V.tensor_tensor(out=sm[:, ADJ + 1], in0=sm[:, MA], in1=sm[:, MA + 1], op=OP.subtract)
    # A12 = h01 h02 - h12 h00
    V.tensor_tensor(out=sm[:, MB], in0=h(3), in1=h(5), op=OP.mult)
    V.tensor_tensor(out=sm[:, MB + 1], in0=h(4), in1=h(0), op=OP.mult)
    V.tensor_tensor(out=sm[:, ADJ + 4], in0=sm[:, MB], in1=sm[:, MB + 1], op=OP.subtract)

    # det = h00 A00 + h01 A01 + h02 A02 ; A(bf16) = adj * (-1/det)
    V.tensor_tensor(out=sm[:, DET], in0=h(0), in1=sm[:, ADJ], op=OP.mult)
    V.tensor_tensor(out=sm[:, MA], in0=h(3), in1=sm[:, ADJ + 1], op=OP.mult)
    V.tensor_tensor(out=sm[:, DET], in0=sm[:, DET], in1=sm[:, MA], op=OP.add)
    V.tensor_tensor(out=sm[:, MB], in0=h(5), in1=sm[:, ADJ + 2], op=OP.mult)
    V.tensor_tensor(out=sm[:, DET], in0=sm[:, DET], in1=sm[:, MB], op=OP.add)
    V.reciprocal(out=sm[:, RDET], in_=sm[:, DET])
    V.scalar_tensor_tensor(out=abf[:], in0=sm[:, ADJ:ADJ + 6], scalar=-1.0,
                           in1=sm[:, RDET].unsqueeze(1).broadcast_to((P, 6, J)),
                           op0=OP.mult, op1=OP.mult)

    # spread A upper-tri -> row-major 3x3 (DVE copies; cheap, avoids Act stall)
    V.tensor_copy(out=a9[:, 0:3], in_=abf[:, 0:3])
    V.tensor_copy(out=a9[:, 3], in_=abf[:, 1])
    V.tensor_copy(out=a9[:, 4:6], in_=abf[:, 3:5])
    V.tensor_copy(out=a9[:, 6], in_=abf[:, 2])
    V.tensor_copy(out=a9[:, 7:9], in_=abf[:, 4:6])

    # ---- bot = A @ Hth, top = -(SM/TM) jac + c x bot, per J-half ---------
    for hj, hjs in ((0, slice(0, JH)), (1, slice(JH, J))):
        SMnb = ctile[:, 5, :, hjs].unsqueeze(1).broadcast_to((P, 3, A, JH))
        m9v = m9[:, :, :, hjs].rearrange("p (r c) a j -> p r c a j", r=3)
        a9v = a9[:, :, hjs].rearrange("p (r c) j -> p r c j", r=3).unsqueeze(3) \
            .broadcast_to((P, 3, 3, A, JH))
        hthv = hth[:, :, :, hjs].unsqueeze(1).broadcast_to((P, 3, 3, A, JH))
        V.tensor_tensor(out=m9v, in0=a9v, in1=hthv, op=OP.mult)
        V.tensor_tensor(out=bot5[:, 0:3, :, hjs], in0=m9[:, 0:9:3, :, hjs],
                        in1=m9[:, 1:9:3, :, hjs], op=OP.add)
        V.tensor_tensor(out=bot5[:, 0:3, :, hjs], in0=bot5[:, 0:3, :, hjs],
                        in1=m9[:, 2:9:3, :, hjs], op=OP.add)
        V.tensor_copy(out=bot5[:, 3:5, :, hjs], in_=bot5[:, 0:2, :, hjs])
        cbb1 = cb5[:, 1:4, hjs].unsqueeze(2).broadcast_to((P, 3, A, JH))
        cbb2 = cb5[:, 2:5, hjs].unsqueeze(2).broadcast_to((P, 3, A, JH))
        V.tensor_tensor(out=scr1[:, :, :, hjs], in0=cbb1,
                        in1=bot5[:, 2:5, :, hjs], op=OP.mult)
        V.tensor_tensor(out=scr2[:, :, :, hjs], in0=cbb2,
                        in1=bot5[:, 1:4, :, hjs], op=OP.mult)
        V.tensor_tensor(out=scr1[:, :, :, hjs], in0=scr1[:, :, :, hjs],
                        in1=scr2[:, :, :, hjs], op=OP.subtract)
        V.tensor_tensor(out=scr2[:, :, :, hjs], in0=SMnb,
                        in1=jac5[:, 0:3, :, hjs], op=OP.mult)
        V.tensor_tensor(out=top3[:, :, :, hjs], in0=scr1[:, :, :, hjs],
                        in1=scr2[:, :, :, hjs], op=OP.add)
        nc.sync.dma_start(out=dram["out"][:, hj, 0:3], in_=top3[:, :, :, hjs])
        nc.sync.dma_start(out=dram["out"][:, hj, 3:6], in_=bot5[:, 0:3, :, hjs])


@functools.lru_cache(maxsize=1)
def _program():
    from contextlib import ExitStack
    import concourse.bacc as bacc
    import concourse.tile as tile
    from concourse import mybir

    f32 = mybir.dt.float32
    b16 = mybir.dt.bfloat16
    nc = bacc.Bacc("TRN2", target_bir_lowering=False, debug=False)
    JH = J // 2
    dram = {
        "rot": nc.dram_tensor("rot", [P, 2, 3, A, JH], b16, kind="ExternalInput"),
        "pos": nc.dram_tensor("pos", [P, 2, 3, A, JH], b16, kind="ExternalInput"),
        "com": nc.dram_tensor("com", [P, 2, 3, A, JH], b16, kind="ExternalInput"),
        "ctm": nc.dram_tensor("ctm", [P, 1, A, J], b16, kind="ExternalInput"),
        "ctr": nc.dram_tensor("ctr", [P, 5, A, J], b16, kind="ExternalInput"),
        "c32": nc.dram_tensor("c32", [P, NC32], f32, kind="ExternalInput"),
        "out": nc.dram_tensor("out", [P, 2, 6, A, JH], b16, kind="ExternalOutput"),
    }
    with tile.TileContext(nc) as tc:
        with ExitStack() as ctx:
            _emit(nc, tc, ctx, dram)
    nc.compile()
    return nc


def pack_inputs(com_list, link_pose_list):
    """Host-side layout packing (pure data movement + dtype cast)."""
    N = N_CORES * P * J
    pose = np.ascontiguousarray(link_pose_list, dtype=np.float32).reshape(N, 4, 4, 9)
    com = np.ascontiguousarray(com_list, dtype=np.float32).reshape(N, 3, 7)
    rot = pose[:, :3, AXIS, np.arange(7)]                # (N, 3, 7)
    pos = pose[:, :3, 3, :7]                             # (N, 3, 7)

    def to_core_layout(x):  # (N, 3, 7) -> (cores, P, 2, 3, 7, JH) J-half chunked
        x = x.reshape(N_CORES, P, J, 3, 7).transpose(0, 1, 3, 4, 2)   # (.., 3, 7, J)
        x = x.reshape(N_CORES, P, 3, 7, 2, J // 2).transpose(0, 1, 4, 2, 3, 5)
        return np.ascontiguousarray(x).astype(BF)

    return to_core_layout(rot), to_core_layout(pos), to_core_layout(com)


def make_in_maps(rot, pos, com):
    ctm = np.ascontiguousarray(np.broadcast_to(CT[0:1], (P, 1, A, J)))
    ctr = np.ascontiguousarray(np.broadcast_to(CT[1:6], (P, 5, A, J)))
    c32 = np.broadcast_to(CONSTS32, (P, NC32)).copy()
    return [
        {"rot": rot[k], "pos": pos[k], "com": com[k], "ctm": ctm, "ctr": ctr,
         "c32": c32}
        for k in range(N_CORES)
    ]


def unpack_output(res):
    out = np.stack([res.results[k]["out"] for k in range(N_CORES)])  # (8,P,2,6,7,JH)
    out = out.astype(np.float32).transpose(0, 1, 2, 5, 3, 4)         # (8,P,2,JH,6,7)
    return np.ascontiguousarray(out.reshape(512, 256, 6, 7))


def _kernel_bm0(com, pose):
    # bm=0 path (not exercised by the shipped setup_inputs; numpy fallback)
    rot = pose[:, :, :3, 2, :N_ACT].copy()
    rot[..., 1] = pose[:, :, :3, 0, 1]
    rot[..., 5] = pose[:, :, :3, 0, 5]
    rot[..., 4] *= -1.0
    delp = pose[:, :, :3, 3, -2][..., None] - pose[:, :, :3, 3, :N_ACT]
    jt = np.cross(rot, delp, axis=2)
    return np.concatenate([jt, rot], axis=2).astype(np.float32)


def kernel(com_list, link_pose_list, bm):
    if not int(bm):
        return _kernel_bm0(np.asarray(com_list, np.float32),
                           np.asarray(link_pose_list, np.float32))

    from concourse.bass_utils import run_bass_kernel_spmd

    nc = _program()
    rot, pos, com = pack_inputs(com_list, link_pose_list)
    res = run_bass_kernel_spmd(nc, make_in_maps(rot, pos, com),
                               core_ids=list(range(N_CORES)))
    return unpack_output(res)


# revision 27
# speedup vs baseline: 1.7406x; 1.0421x over previous
"""Trainium2 Bass kernel for nn_CanadarmJacob (space-arm Jacobian, bm=1 path).

Contract: kernel(**inputs) takes FULL inputs (com_list (512,256,3,7) f32,
link_pose_list (512,256,4,4,9) f32, bm scalar) and returns the FULL output
(512,256,6,7) f32. Internally shards samples across 8 NeuronCores (pure data
parallel), runs a Bass/Tile kernel per core, and gathers.

v2 design: bf16 streams, act-major layout (P, comp, act, J) with J contiguous
so every big op hits the DVE 2x bf16 mode (0.52 ns/elem). Host packs only the
needed pose slices (rot gather + pos) -> 4.4x less input DMA. 3x3 smalls chain
stays f32. Activation engine carries the affine/copy side-channel (sign flip,
row duplication for shifted cross-product views, dtype casts, A-matrix spread).

Math (reformulated from the reference):
  rot   = pose[:3, AXIS[a], a], AXIS=[2,0,2,2,2,0,2]; rot[:,4] *= -1
  del   = com - pos ;  mdel = M del ; mcom = M com
  u     = {mdel_i del_j} (6) ; S = sum_a u ; scom = sum_a mcom (pairwise trees)
  w     = suffix-cumsum_a(mdel) ; jac = rot x del
  c     = scom/TM - BASE ; w2 = w - SM (x) c
  Hth   = D_suf . rot + w2 x jac ; jsm = SM jac
  H_s   = TM(cc^T - |c|^2 I) + diag(CD) + (trS) I - S   (3x3 symmetric)
  A     = -inv(H_s) via adjugate ; bot = A @ Hth ; top = -jsm/TM + c x bot
"""
import sys
import functools

if "/opt/trn_rl_repo" not in sys.path:
    sys.path.insert(0, "/opt/trn_rl_repo")

import numpy as np
import ml_dtypes

BF = ml_dtypes.bfloat16

# ---------------------------------------------------------------- constants
N_CORES = 8
P = 128          # SBUF partitions
J = 128          # samples per partition per core
A = 7            # actuated links
N_ACT = 7

AXIS = np.array([2, 0, 2, 2, 2, 0, 2])
MASS = np.array([105.98, 105.98, 314.98, 279.2, 105.98, 105.98, 243.66], np.float64)
TM = float(MASS.sum() + 100000.0 + 243.66)
DIAGS = np.array([[12.19, 12.19, 3.061], [12.19, 12.19, 3.061], [15.41, 2094.71, 2103.19],
                  [9.522, 1966.28, 1966.28], [8.305, 3.061, 8.0386], [12.13, 12.13, 3.061],
                  [9.336, 44.41, 44.41]], np.float64)
D_SUF = np.cumsum(DIAGS[::-1], axis=0)[::-1]          # (7,3) suffix inertia diag
SM = np.cumsum(MASS[::-1])[::-1]                      # (7,) suffix mass
CD = DIAGS.sum(axis=0)                                # (3,)
_TF0 = np.array([[1, 0, 0, 0], [0, -1, 0, 0], [0, 0, 1.3, 6], [0, 0, 0, 1]], np.float64)
_COM0 = np.array([[1, 0, 0, 0], [0, 1, 0, 0], [0, 0, 1, 0.5], [0, 0, 0, 1]], np.float64)
BASE = (_TF0 @ _COM0)[:3, 3] * 243.66 / (100000.0 + 243.66)   # [0, 0, ~0.0162]

# ctile rows (bf16, each (A, J) broadcast over J):
#   0=M, 1=SM, 2..4=D_suf[c], 5=-SM/TM
CT = np.broadcast_to(
    np.concatenate([MASS[None, :], SM[None, :], D_SUF.T,
                    (-SM / TM)[None, :]]).astype(np.float32)[:, :, None],
    (6, A, J)).astype(BF)
# f32 per-partition consts row: CD (3)
CONSTS32 = np.array(list(CD) + [float(BASE[2])], np.float32)
NC32 = CONSTS32.shape[0]


def _emit(nc, tc, ctx, dram):
    from concourse import mybir

    f32 = mybir.dt.float32
    b16 = mybir.dt.bfloat16
    OP = mybir.AluOpType
    V = nc.vector
    SE = nc.scalar           # Activation engine
    Copy = mybir.ActivationFunctionType.Copy

    pool = ctx.enter_context(tc.tile_pool(name="main", bufs=1))

    # ---- tiles (act-major: last dim J contiguous) -------------------------
    ctile = pool.tile([P, 6, A, J], b16)       # M, SM, Dx, Dy, Dz, -SM/TM
    c32 = pool.tile([P, NC32], f32)
    rot5 = pool.tile([P, 5, A, J], b16)        # rows 0-2 rot, 3-4 dup(x,y)
    pos = pool.tile([P, 3, A, J], b16)
    com = pool.tile([P, 3, A, J], b16)
    del5 = pool.tile([P, 5, A, J], b16)
    mdel = pool.tile([P, 3, A, J], b16)        # becomes w in place (suffix cumsum)
    prods = pool.tile([P, 9, A, J], b16)       # u rows 0-5 [xx,yy,zz,xy,yz,xz], mcom 6-8
    tl1 = pool.tile([P, 9, 3, J], b16)         # tree L1
    tc0 = pool.tile([P, 9, J], b16)            # tree L2 left
    tc1 = pool.tile([P, 9, J], b16)            # tree L2 right
    sums = pool.tile([P, 9, J], b16)           # S rows 0-5, scom rows 6-8
    jac5 = pool.tile([P, 5, A, J], b16)
    scr1 = pool.tile([P, 3, A, J], b16)
    scr2 = pool.tile([P, 3, A, J], b16)
    smc = pool.tile([P, 3, A, J], b16)
    w25 = pool.tile([P, 5, A, J], b16)
    hth = pool.tile([P, 3, A, J], b16)
    m9 = pool.tile([P, 9, A, J], b16)
    bot5 = pool.tile([P, 5, A, J], b16)
    outt = pool.tile([P, 2, 6, A, J // 2], b16)
    c5 = pool.tile([P, 5, J], f32)             # c rows x,y,z,x,y (f32)
    cb5 = pool.tile([P, 5, J], b16)            # c in bf16 + dup
    sm = pool.tile([P, 26, J], f32)            # smalls scratch
    a9 = pool.tile([P, 9, J], b16)             # A row-major 3x3

    # smalls row map (sm tile)
    CC = 0      # rows 0-2 diag(xx,yy,zz), 3-5 off (xy,yz,xz)
    CSQ = 6
    SSR = 7
    T3 = 8      # rows 8-10
    D3 = 11     # rows 11-13
    HS = 14     # rows 14-19: [h00,h11,h22,h01,h12,h02]
    ADJ = 20    # rows 20-25 order [A00,A01,A02,A11,A12,A22]
    MA = 8      # scratch pair rows 8-9 (T3 dead after HS built)
    MB = 10     # scratch pair rows 10-11
    DET = 12
    RDET = 13

    # ---- input DMAs (pos/com first so compute starts early) --------------
    JH = J // 2
    nc.sync.dma_start(out=pos[:], in_=dram["pos"][:])
    nc.sync.dma_start(out=com[:], in_=dram["com"][:])
    nc.sync.dma_start(out=ctile[:, 0:1], in_=dram["ctm"][:])
    nc.sync.dma_start(out=rot5[:, 0:3], in_=dram["rot"][:])
    nc.sync.dma_start(out=ctile[:, 1:6], in_=dram["ctr"][:])
    nc.sync.dma_start(out=c32[:], in_=dram["c32"][:])

    SE.mul(rot5[:, 0:3, 4], rot5[:, 0:3, 4], -1.0)
    SE.copy(rot5[:, 3:5], rot5[:, 0:2])

    Mb = ctile[:, 0].unsqueeze(1).broadcast_to((P, 3, A, J))
    SMb = ctile[:, 1].unsqueeze(1).broadcast_to((P, 3, A, J))
    Dt = ctile[:, 2:5]

    # ---- streams (DVE unless noted) --------------------------------------
    V.tensor_tensor(out=del5[:, 0:3], in0=com[:], in1=pos[:], op=OP.subtract)
    SE.copy(del5[:, 3:5], del5[:, 0:2])
    V.tensor_tensor(out=mdel[:], in0=Mb, in1=del5[:, 0:3], op=OP.mult)
    V.tensor_tensor(out=prods[:, 6:9], in0=Mb, in1=com[:], op=OP.mult)
    V.tensor_tensor(out=prods[:, 0:3], in0=mdel[:], in1=del5[:, 0:3], op=OP.mult)
    V.tensor_tensor(out=prods[:, 3:5], in0=mdel[:, 0:2], in1=del5[:, 1:3], op=OP.mult)
    V.tensor_tensor(out=prods[:, 5], in0=mdel[:, 0], in1=del5[:, 2], op=OP.mult)

    # pairwise act-sum tree over prods: (7) -> S rows 0-5, scom rows 6-8
    V.tensor_tensor(out=tl1[:], in0=prods[:, :, 0:3], in1=prods[:, :, 4:7], op=OP.add)
    V.tensor_tensor(out=tc0[:], in0=tl1[:, :, 0], in1=tl1[:, :, 1], op=OP.add)
    V.tensor_tensor(out=tc1[:], in0=tl1[:, :, 2], in1=prods[:, :, 3], op=OP.add)
    V.tensor_tensor(out=sums[:], in0=tc0[:], in1=tc1[:], op=OP.add)

    # jac = rot x del via shifted dup views
    V.tensor_tensor(out=scr1[:], in0=rot5[:, 1:4], in1=del5[:, 2:5], op=OP.mult)
    V.tensor_tensor(out=scr2[:], in0=rot5[:, 2:5], in1=del5[:, 1:4], op=OP.mult)
    V.tensor_tensor(out=jac5[:, 0:3], in0=scr1[:], in1=scr2[:], op=OP.subtract)
    SE.copy(jac5[:, 3:5], jac5[:, 0:2])

    # w: suffix cumsum over acts, in place in mdel
    for k in range(A - 2, -1, -1):
        V.tensor_tensor(out=mdel[:, :, k], in0=mdel[:, :, k], in1=mdel[:, :, k + 1],
                        op=OP.add)

    # c = scom/TM - BASE (Act engine), then bf16 copy + dups
    SE.mul(c5[:, 0:2], sums[:, 6:8], 1.0 / TM)
    SE.activation(c5[:, 2], sums[:, 8], Copy, bias=-float(BASE[2]), scale=1.0 / TM)
    SE.copy(c5[:, 3:5], c5[:, 0:2])
    SE.copy(cb5[:, 0:3], c5[:, 0:3])
    SE.copy(cb5[:, 3:5], cb5[:, 0:2])

    # w2 = w - SM (x) c
    cbb = cb5[:, 0:3].unsqueeze(2).broadcast_to((P, 3, A, J))
    V.tensor_tensor(out=smc[:], in0=SMb, in1=cbb, op=OP.mult)
    V.tensor_tensor(out=w25[:, 0:3], in0=mdel[:], in1=smc[:], op=OP.subtract)
    SE.copy(w25[:, 3:5], w25[:, 0:2])

    # Hth = D.rot + w2 x jac
    V.tensor_tensor(out=smc[:], in0=Dt, in1=rot5[:, 0:3], op=OP.mult)
    V.tensor_tensor(out=scr1[:], in0=w25[:, 1:4], in1=jac5[:, 2:5], op=OP.mult)
    V.tensor_tensor(out=scr2[:], in0=w25[:, 2:5], in1=jac5[:, 1:4], op=OP.mult)
    V.tensor_tensor(out=scr1[:], in0=scr1[:], in1=scr2[:], op=OP.subtract)
    V.tensor_tensor(out=hth[:], in0=smc[:], in1=scr1[:], op=OP.add)

    # ---- smalls (f32) ----------------------------------------------------
    # cc products via dup'd c5: diag then off [xy,yz,xz]
    V.tensor_tensor(out=sm[:, CC:CC + 3], in0=c5[:, 0:3], in1=c5[:, 0:3], op=OP.mult)
    V.tensor_tensor(out=sm[:, CC + 3:CC + 6], in0=c5[:, 0:3], in1=c5[:, 1:4], op=OP.mult)
    V.tensor_tensor(out=sm[:, CSQ], in0=sm[:, CC], in1=sm[:, CC + 1], op=OP.add)
    V.tensor_tensor(out=sm[:, CSQ], in0=sm[:, CSQ], in1=sm[:, CC + 2], op=OP.add)
    V.tensor_tensor(out=sm[:, SSR], in0=sums[:, 0], in1=sums[:, 1], op=OP.add)
    V.tensor_tensor(out=sm[:, SSR], in0=sm[:, SSR], in1=sums[:, 2], op=OP.add)

    CDb = c32[:, 0:3].unsqueeze(2).broadcast_to((P, 3, J))
    ssb = sm[:, SSR].unsqueeze(1).broadcast_to((P, 3, J))
    csqb = sm[:, CSQ].unsqueeze(1).broadcast_to((P, 3, J))
    V.tensor_tensor(out=sm[:, T3:T3 + 3], in0=CDb, in1=sums[:, 0:3], op=OP.subtract)
    V.tensor_tensor(out=sm[:, T3:T3 + 3], in0=sm[:, T3:T3 + 3], in1=ssb, op=OP.add)
    V.tensor_tensor(out=sm[:, D3:D3 + 3], in0=sm[:, CC:CC + 3], in1=csqb, op=OP.subtract)
    V.scalar_tensor_tensor(out=sm[:, HS:HS + 3], in0=sm[:, D3:D3 + 3], scalar=TM,
                           in1=sm[:, T3:T3 + 3], op0=OP.mult, op1=OP.add)
    V.scalar_tensor_tensor(out=sm[:, HS + 3:HS + 6], in0=sm[:, CC + 3:CC + 6],
                           scalar=TM, in1=sums[:, 3:6], op0=OP.mult, op1=OP.subtract)

    # adjugate of Hs rows [h00,h11,h22,h01,h12,h02] -> adj [A00,A01,A02,A11,A12,A22]
    h = lambda i: sm[:, HS + i]
    m2a = sm[:, MA:MA + 2]
    m2b = sm[:, MB:MB + 2]
    # A00 = h11 h22 - h12^2 ; A11 = h00 h22 - h02^2
    V.tensor_tensor(out=m2a, in0=sm[:, HS + 1:HS - 1:-1],
                    in1=h(2).unsqueeze(1).broadcast_to((P, 2, J)), op=OP.mult)
    V.tensor_tensor(out=m2b, in0=sm[:, HS + 4:HS + 6],
                    in1=sm[:, HS + 4:HS + 6], op=OP.mult)
    V.tensor_tensor(out=sm[:, ADJ:ADJ + 4:3], in0=m2a, in1=m2b, op=OP.subtract)
    # A22 = h00 h11 - h01^2 ; A02 = h01 h12 - h02 h11
    V.tensor_tensor(out=m2a, in0=sm[:, HS:HS + 4:3], in1=sm[:, HS + 1:HS + 5:3],
                    op=OP.mult)
    V.tensor_tensor(out=m2b, in0=sm[:, HS + 3:HS + 6:2], in1=sm[:, HS + 3:HS - 1:-2],
                    op=OP.mult)
    V.tensor_tensor(out=sm[:, ADJ + 5:ADJ + 1:-3], in0=m2a, in1=m2b, op=OP.subtract)
    # A01 = h02 h12 - h01 h22
    V.tensor_tensor(out=sm[:, MA], in0=h(5), in1=h(4), op=OP.mult)
    V.tensor_tensor(out=sm[:, MA + 1], in0=h(3), in1=h(2), op=OP.mult)
    V.tensor_tensor(out=sm[:, ADJ + 1], in0=sm[:, MA], in1=sm[:, MA + 1], op=OP.subtract)
    # A12 = h01 h02 - h12 h00
    V.tensor_tensor(out=sm[:, MB], in0=h(3), in1=h(5), op=OP.mult)
    V.tensor_tensor(out=sm[:, MB + 1], in0=h(4), in1=h(0), op=OP.mult)
    V.tensor_tensor(out=sm[:, ADJ + 4], in0=sm[:, MB], in1=sm[:, MB + 1], op=OP.subtract)

    # det = h00 A00 + h01 A01 + h02 A02 ; A(bf16) = adj * (-1/det)
    V.tensor_tensor(out=sm[:, DET], in0=h(0), in1=sm[:, ADJ], op=OP.mult)
    V.tensor_tensor(out=sm[:, MA], in0=h(3), in1=sm[:, ADJ + 1], op=OP.mult)
    V.tensor_tensor(out=sm[:, DET], in0=sm[:, DET], in1=sm[:, MA], op=OP.add)
    V.tensor_tensor(out=sm[:, MB], in0=h(5), in1=sm[:, ADJ + 2], op=OP.mult)
    V.tensor_tensor(out=sm[:, DET], in0=sm[:, DET], in1=sm[:, MB], op=OP.add)
    V.reciprocal(out=sm[:, RDET], in_=sm[:, DET])
    # A = adj * (-1/det) as bf16, written straight into row-major 3x3 rows
    # [00,01,02,10,11,12,20,21,22]: upper entries by STT, mirrors by copy
    V.scalar_tensor_tensor(out=a9[:, 0:3], in0=sm[:, ADJ:ADJ + 3], scalar=-1.0,
                           in1=sm[:, RDET].unsqueeze(1).broadcast_to((P, 3, J)),
                           op0=OP.mult, op1=OP.mult)
    V.scalar_tensor_tensor(out=a9[:, 4:6], in0=sm[:, ADJ + 3:ADJ + 5], scalar=-1.0,
                           in1=sm[:, RDET].unsqueeze(1).broadcast_to((P, 2, J)),
                           op0=OP.mult, op1=OP.mult)
    V.scalar_tensor_tensor(out=a9[:, 8], in0=sm[:, ADJ + 5], scalar=-1.0,
                           in1=sm[:, RDET], op0=OP.mult, op1=OP.mult)
    V.tensor_copy(out=a9[:, 3], in_=a9[:, 1])
    V.tensor_copy(out=a9[:, 6], in_=a9[:, 2])
    V.tensor_copy(out=a9[:, 7], in_=a9[:, 5])

    # ---- bot = A @ Hth (full J: keeps the big product op 3D-mergeable) ----
    m9v = m9[:].rearrange("p (r c) a j -> p r c a j", r=3)
    a9v = a9[:].rearrange("p (r c) j -> p r c j", r=3).unsqueeze(3) \
        .broadcast_to((P, 3, 3, A, J))
    hthv = hth[:].unsqueeze(1).broadcast_to((P, 3, 3, A, J))
    V.tensor_tensor(out=m9v, in0=a9v, in1=hthv, op=OP.mult)

    # ---- bot sums, top = -(SM/TM) jac + c x bot, per J-half --------------
    for hj, hjs in ((0, slice(0, JH)), (1, slice(JH, J))):
        SMnb = ctile[:, 5, :, hjs].unsqueeze(1).broadcast_to((P, 3, A, JH))
        V.tensor_tensor(out=bot5[:, 0:3, :, hjs], in0=m9[:, 0:9:3, :, hjs],
                        in1=m9[:, 1:9:3, :, hjs], op=OP.add)
        V.tensor_tensor(out=bot5[:, 0:3, :, hjs], in0=bot5[:, 0:3, :, hjs],
                        in1=m9[:, 2:9:3, :, hjs], op=OP.add)
        V.tensor_copy(out=bot5[:, 3:5, :, hjs], in_=bot5[:, 0:2, :, hjs])
        V.tensor_copy(out=outt[:, hj, 3:6], in_=bot5[:, 0:3, :, hjs])
        cbb1 = cb5[:, 1:4, hjs].unsqueeze(2).broadcast_to((P, 3, A, JH))
        cbb2 = cb5[:, 2:5, hjs].unsqueeze(2).broadcast_to((P, 3, A, JH))
        V.tensor_tensor(out=scr1[:, :, :, hjs], in0=cbb1,
                        in1=bot5[:, 2:5, :, hjs], op=OP.mult)
        V.tensor_tensor(out=scr2[:, :, :, hjs], in0=cbb2,
                        in1=bot5[:, 1:4, :, hjs], op=OP.mult)
        V.tensor_tensor(out=scr1[:, :, :, hjs], in0=scr1[:, :, :, hjs],
                        in1=scr2[:, :, :, hjs], op=OP.subtract)
        V.tensor_tensor(out=scr2[:, :, :, hjs], in0=SMnb,
                        in1=jac5[:, 0:3, :, hjs], op=OP.mult)
        V.tensor_tensor(out=outt[:, hj, 0:3], in0=scr1[:, :, :, hjs],
                        in1=scr2[:, :, :, hjs], op=OP.add)
        nc.sync.dma_start(out=dram["out"][:, hj], in_=outt[:, hj])


@functools.lru_cache(maxsize=1)
def _program():
    from contextlib import ExitStack
    import concourse.bacc as bacc
    import concourse.tile as tile
    from concourse import mybir

    f32 = mybir.dt.float32
    b16 = mybir.dt.bfloat16
    nc = bacc.Bacc("TRN2", target_bir_lowering=False, debug=False)
    JH = J // 2
    dram = {
        "rot": nc.dram_tensor("rot", [P, 3, A, J], b16, kind="ExternalInput"),
        "pos": nc.dram_tensor("pos", [P, 3, A, J], b16, kind="ExternalInput"),
        "com": nc.dram_tensor("com", [P, 3, A, J], b16, kind="ExternalInput"),
        "ctm": nc.dram_tensor("ctm", [P, 1, A, J], b16, kind="ExternalInput"),
        "ctr": nc.dram_tensor("ctr", [P, 5, A, J], b16, kind="ExternalInput"),
        "c32": nc.dram_tensor("c32", [P, NC32], f32, kind="ExternalInput"),
        "out": nc.dram_tensor("out", [P, 2, 6, A, JH], b16, kind="ExternalOutput"),
    }
    with tile.TileContext(nc) as tc:
        with ExitStack() as ctx:
            _emit(nc, tc, ctx, dram)
    nc.compile()
    return nc


def pack_inputs(com_list, link_pose_list):
    """Host-side layout packing (pure data movement + dtype cast)."""
    N = N_CORES * P * J
    pose = np.ascontiguousarray(link_pose_list, dtype=np.float32).reshape(N, 4, 4, 9)
    com = np.ascontiguousarray(com_list, dtype=np.float32).reshape(N, 3, 7)
    rot = pose[:, :3, AXIS, np.arange(7)]                # (N, 3, 7)
    pos = pose[:, :3, 3, :7]                             # (N, 3, 7)

    def to_core_layout(x):  # (N, 3, 7) -> (cores, P, 3, 7, J)
        return np.ascontiguousarray(
            x.reshape(N_CORES, P, J, 3, 7).transpose(0, 1, 3, 4, 2)).astype(BF)

    return to_core_layout(rot), to_core_layout(pos), to_core_layout(com)


def make_in_maps(rot, pos, com):
    ctm = np.ascontiguousarray(np.broadcast_to(CT[0:1], (P, 1, A, J)))
    ctr = np.ascontiguousarray(np.broadcast_to(CT[1:6], (P, 5, A, J)))
    c32 = np.broadcast_to(CONSTS32, (P, NC32)).copy()
    return [
        {"rot": rot[k], "pos": pos[k], "com": com[k], "ctm": ctm, "ctr": ctr,
         "c32": c32}
        for k in range(N_CORES)
    ]


def unpack_output(res):
    out = np.stack([res.results[k]["out"] for k in range(N_CORES)])  # (8,P,2,6,7,JH)
    out = out.astype(np.float32).transpose(0, 1, 2, 5, 3, 4)         # (8,P,2,JH,6,7)
    return np.ascontiguousarray(out.reshape(512, 256, 6, 7))


def _kernel_bm0(com, pose):
    # bm=0 path (not exercised by the shipped setup_inputs; numpy fallback)
    rot = pose[:, :, :3, 2, :N_ACT].copy()
    rot[..., 1] = pose[:, :, :3, 0, 1]
    rot[..., 5] = pose[:, :, :3, 0, 5]
    rot[..., 4] *= -1.0
    delp = pose[:, :, :3, 3, -2][..., None] - pose[:, :, :3, 3, :N_ACT]
    jt = np.cross(rot, delp, axis=2)
    return np.concatenate([jt, rot], axis=2).astype(np.float32)


def kernel(com_list, link_pose_list, bm):
    if not int(bm):
        return _kernel_bm0(np.asarray(com_list, np.float32),
                           np.asarray(link_pose_list, np.float32))

    from concourse.bass_utils import run_bass_kernel_spmd

    nc = _program()
    rot, pos, com = pack_inputs(com_list, link_pose_list)
    res = run_bass_kernel_spmd(nc, make_in_maps(rot, pos, com),
                               core_ids=list(range(N_CORES)))
    return unpack_output(res)
